# revision 4
# baseline (speedup 1.0000x reference)
"""Multi-head attention (B=4, N=2048, C=768, H=12, D=64) on 8 TRN2 NeuronCores.

Sharding: core c handles batch c//2 and query rows (c%2)*1024 .. +1024, all
heads. Each core recomputes K/V for its full batch; the host ROTATES x[b].T
per core so this core's query rows are always columns 0:1024 (attention is
permutation-invariant over keys), keeping the SPMD graph core-independent.

Pipeline (single TileContext; Tile's priority scheduler interleaves):
  - qkv chains (PE) feed head-pair processing; kT/qT bands for pair p+1 and
    the v tiles are emitted as background PE work inside earlier pairs.
  - Scores for the two heads of a pair are row-tiled (K=64 at PE row groups
    0-1 / 2-3 via base_partition auto tile_position) so both heads' scores
    matmuls run CONCURRENTLY on the PE array -> 2x scores throughput.
  - Softmax exp is split between ScalarE (native EXP) and VectorE (3-op
    Schraudolph: t1=int32(A*s+B); t2=t1+2^22 in the int domain via the f32
    value path; u=(bitcast(t2)*w + bitcast(t1)) -> bf16). The B constant is
    shifted so the DVE variant has unit mean ratio vs true exp (softmax mixes
    ACT- and DVE-produced tiles within one column). A greedy ns-ledger
    balances the two engines, also accounting DVE evict/normalize work.
  - AV matmuls (M=65: [v_h | ones] stationary so PSUM row 64 accumulates the
    softmax denominator) trail the exp stream kt-by-kt, both heads
    interleaved per kt so ut tiles release immediately.
  - Normalization: reciprocal_approx_fast directly on the PSUM sums row,
    gpsimd partition-broadcast, fused into the PSUM->SBUF evict.
The 1/sqrt(D) scale is folded into the q columns of wqkvT on the host.
"""

from contextlib import ExitStack

import ml_dtypes
import numpy as np

import concourse.bass as bass
import concourse.tile as tile
from concourse import bacc, mybir
from concourse import bass_utils

B, N, C, H, Dh = 4, 2048, 768, 12, 64
P = 128
NCORES = 8
ROWS = N // 2  # query rows per core
SCALE = Dh ** -0.5

BF16 = mybir.dt.bfloat16
F32 = mybir.dt.float32
I32 = mybir.dt.int32

CB = C // P       # 6 contraction bands
NT = N // P       # 16 key tiles
NPAIR = H // 2    # 6 head pairs

# DVE Schraudolph-exp constants (see docstring; B pre-shifted for unit mean)
EXP_A = float(2 ** 23 * 1.4426950408889634)
EXP_B = 1056414412.88
EXP_W = 0.7150368

# emission-time ns ledger costs for the exp/evict balance
ACT_EXP_NS = 1160.0
DVE_EXP_NS = 2400.0
DVE_EV1024_NS = 900.0
DVE_EV768_NS = 800.0
DVE_NORM_NS = 1550.0  # recip + mul

_cached_nc = None
LAST_RESULT = None  # BassKernelResults of the most recent run (for test harness)


def _build_nc():
    nc = bacc.Bacc(
        "TRN2",
        target_bir_lowering=False,
        debug=False,
        enable_asserts=False,
        num_devices=NCORES,
    )
    xT_d = nc.dram_tensor("xT", [C, N], BF16, kind="ExternalInput")
    wqkvT_d = nc.dram_tensor("wqkvT", [C, 3 * C], BF16, kind="ExternalInput")
    wprojT_d = nc.dram_tensor("wprojT", [C, C], BF16, kind="ExternalInput")
    bproj_d = nc.dram_tensor("bproj", [CB, P, 1], F32, kind="ExternalInput")
    out_d = nc.dram_tensor("out", [C, ROWS], F32, kind="ExternalOutput")

    Exp = mybir.ActivationFunctionType.Exp
    Op = mybir.AluOpType

    ledger = {"act": 0.0, "dve": 0.0}

    with tile.TileContext(nc) as tc:
        with ExitStack() as ctx:
            # ---- persistent pools ----
            pool_wp = ctx.enter_context(tc.tile_pool(name="wproj", bufs=1))
            pool_bias = ctx.enter_context(tc.tile_pool(name="bias", bufs=1))
            pool_qT = ctx.enter_context(tc.tile_pool(name="qT", bufs=1))
            pool_kT = ctx.enter_context(tc.tile_pool(name="kT", bufs=1))
            pool_vo = ctx.enter_context(tc.tile_pool(name="vones", bufs=1))
            pool_attT = ctx.enter_context(tc.tile_pool(name="attT", bufs=1))
            pool_ut = ctx.enter_context(tc.tile_pool(name="ut", bufs=16))
            pool_t1 = ctx.enter_context(tc.tile_pool(name="t1", bufs=2))
            pool_t2 = ctx.enter_context(tc.tile_pool(name="t2", bufs=2))
            pool_r = ctx.enter_context(tc.tile_pool(name="r", bufs=2))
            pool_rb = ctx.enter_context(tc.tile_pool(name="rb", bufs=2))
            pool_y = ctx.enter_context(tc.tile_pool(name="y", bufs=3))
            ps_mix = ctx.enter_context(tc.tile_pool(name="ps_mix", bufs=2, space="PSUM"))
            ps_pu = ctx.enter_context(tc.tile_pool(name="ps_pu", bufs=2, space="PSUM"))

            wp_sb = [pool_wp.tile([P, C], BF16, name=f"wp{i}") for i in range(CB)]
            bias_sb = [pool_bias.tile([P, 1], F32, name=f"bias{i}") for i in range(CB)]
            qT_sb = [pool_qT.tile([P, ROWS], BF16, name=f"qT{i}") for i in range(CB)]
            kT_sb = [pool_kT.tile([P, N], BF16, name=f"kT{i}") for i in range(CB)]
            # per key-tile: 12 heads x [v_h (64 cols) | ones (1 col)]
            vo_sb = [pool_vo.tile([P, H * (Dh + 1)], BF16, name=f"vo{i}") for i in range(NT)]
            attT_sb = [pool_attT.tile([P, ROWS], BF16, name=f"attT{i}") for i in range(CB)]

            # ---- stage-1 transient pools (closed after last qkv chain) ----
            s1 = ExitStack()
            pool_x = s1.enter_context(tc.tile_pool(name="xT", bufs=1))
            pool_wqkv = s1.enter_context(tc.tile_pool(name="wqkv", bufs=1))
            x_sb = [pool_x.tile([P, N], BF16, name=f"x{i}") for i in range(CB)]
            wqkv_sb = [pool_wqkv.tile([P, 3 * C], BF16, name=f"wqkv{i}") for i in range(CB)]

            # DMA priority: x q-cols + band-0 weights, then x k-cols, then the
            # remaining qk weight bands, then v weights, then proj weights.
            for cb in range(CB):
                nc.sync.dma_start(x_sb[cb][:, 0:ROWS], xT_d.ap()[cb * P:(cb + 1) * P, 0:ROWS])
                nc.sync.dma_start(wqkv_sb[cb][:, 0:P], wqkvT_d.ap()[cb * P:(cb + 1) * P, 0:P])
                nc.sync.dma_start(wqkv_sb[cb][:, C:C + P], wqkvT_d.ap()[cb * P:(cb + 1) * P, C:C + P])
            for cb in range(CB):
                nc.sync.dma_start(x_sb[cb][:, ROWS:N], xT_d.ap()[cb * P:(cb + 1) * P, ROWS:N])
            for cb in range(CB):
                nc.sync.dma_start(wqkv_sb[cb][:, P:C], wqkvT_d.ap()[cb * P:(cb + 1) * P, P:C])
                nc.sync.dma_start(wqkv_sb[cb][:, C + P:2 * C], wqkvT_d.ap()[cb * P:(cb + 1) * P, C + P:2 * C])
            for cb in range(CB):
                nc.sync.dma_start(wqkv_sb[cb][:, 2 * C:3 * C], wqkvT_d.ap()[cb * P:(cb + 1) * P, 2 * C:3 * C])
            for cb in range(CB):
                nc.sync.dma_start(wp_sb[cb][:], wprojT_d.ap()[cb * P:(cb + 1) * P, :])
                nc.sync.dma_start(bias_sb[cb][:], bproj_d.ap()[cb, :, :])

            for nt in range(NT):
                nc.gpsimd.memset(vo_sb[nt][:], 1.0)

            # ---- qkv projection chains (PE) ----
            def gen_qT(b):
                pt = ps_mix.tile([P, 1024], F32, name="ptq", tag="mix")
                for cb in range(CB):  # cb outer: both qc halves share one stationary
                    for qc in range(2):
                        nc.tensor.matmul(
                            pt[:, qc * 512:(qc + 1) * 512],
                            wqkv_sb[cb][:, b * P:(b + 1) * P],
                            x_sb[cb][:, qc * 512:(qc + 1) * 512],
                            start=(cb == 0), stop=(cb == CB - 1),
                        )
                nc.vector.tensor_copy(qT_sb[b][:], pt[:])
                ledger["dve"] += DVE_EV1024_NS
                yield

            def gen_kT(b):
                for kc in range(2):
                    pt = ps_mix.tile([P, 1024], F32, name="ptk", tag="mix")
                    for cb in range(CB):
                        for half in range(2):
                            nc.tensor.matmul(
                                pt[:, half * 512:(half + 1) * 512],
                                wqkv_sb[cb][:, C + b * P:C + (b + 1) * P],
                                x_sb[cb][:, kc * 1024 + half * 512:kc * 1024 + (half + 1) * 512],
                                start=(cb == 0), stop=(cb == CB - 1),
                            )
                    nc.vector.tensor_copy(kT_sb[b][:, kc * 1024:(kc + 1) * 1024], pt[:])
                    ledger["dve"] += DVE_EV1024_NS
                    yield

            def gen_v(nt):
                pt = ps_mix.tile([P, C], F32, name="ptv", tag="mix")
                for cb in range(CB):  # cb outer: x-tile stationary shared by chunks
                    for off, width in ((0, 512), (512, 256)):  # bank-aligned
                        nc.tensor.matmul(
                            pt[:, off:off + width],
                            x_sb[cb][:, nt * P:(nt + 1) * P],
                            wqkv_sb[cb][:, 2 * C + off:2 * C + off + width],
                            start=(cb == 0), stop=(cb == CB - 1),
                        )
                nc.vector.tensor_copy(
                    vo_sb[nt].rearrange("p (h e) -> p h e", e=Dh + 1)[:, :, 0:Dh],
                    pt[:].rearrange("p (h e) -> p h e", e=Dh),
                )
                ledger["dve"] += DVE_EV768_NS
                yield

            # ---- stage 2 ----
            def emit_exp(ps):
                ut = pool_ut.tile([P, 1024], BF16, name="ut")
                if True or ledger["act"] + ACT_EXP_NS <= ledger["dve"] + DVE_EXP_NS:
                    nc.scalar.activation(ut[:], ps[:], Exp)
                    ledger["act"] += ACT_EXP_NS
                else:
                    t1 = pool_t1.tile([P, 1024], I32, name="t1")
                    t2 = pool_t2.tile([P, 1024], I32, name="t2")
                    nc.vector.tensor_scalar(t1[:], ps[:], EXP_A, EXP_B, Op.mult, Op.add)
                    nc.vector.tensor_scalar(t2[:], t1[:], float(2 ** 22), None, Op.add)
                    nc.vector.scalar_tensor_tensor(
                        ut[:], t2[:].bitcast(F32), EXP_W, t1[:].bitcast(F32),
                        Op.mult, Op.add)
                    ledger["dve"] += DVE_EXP_NS
                return ut

            def gen_scores(p, uts):
                for kt in range(NT):
                    for qc in range(2):
                        ps = ps_mix.tile([P, 1024], F32, name="sc", tag="mix")
                        for hp in range(2):
                            nc.tensor.matmul(
                                ps[:, hp * 512:(hp + 1) * 512],
                                kT_sb[p][hp * 64:hp * 64 + 64, kt * P:(kt + 1) * P],
                                qT_sb[p][hp * 64:hp * 64 + 64, qc * 512:(qc + 1) * 512],
                                start=True, stop=True,
                            )
                        uts[(kt, qc)] = emit_exp(ps)
                        yield

            def normalize(h, pu):
                band, hp = divmod(h, 2)
                po = hp * 64
                s = pool_r.tile([1, ROWS], F32, name="s", tag="r")
                nc.vector.tensor_copy(s[:], pu[64:65, :])
                r = pool_r.tile([1, ROWS], F32, name="r", tag="r")
                nc.vector.reciprocal_approx_fast(r[:], s[:])
                rb = pool_rb.tile([64, ROWS], F32, name="rb")
                nc.gpsimd.partition_broadcast(rb[:], r[:])
                nc.vector.tensor_mul(attT_sb[band][po:po + 64, :], pu[0:64, :], rb[:])
                ledger["dve"] += DVE_NORM_NS

            def gen_av(p, uts):
                pus = [ps_pu.tile([P, 1024], F32, name=f"pu{hp}", tag="pu") for hp in range(2)]
                for kt in range(NT):
                    for hp in range(2):
                        h = 2 * p + hp
                        for qc in range(2):
                            nc.tensor.matmul(
                                pus[hp][0:65, qc * 512:(qc + 1) * 512],
                                vo_sb[kt][:, h * 65:(h + 1) * 65],
                                uts[(kt, qc)][:, hp * 512:(hp + 1) * 512],
                                start=(kt == 0), stop=(kt == NT - 1),
                            )
                    yield
                for hp in range(2):
                    normalize(2 * p + hp, pus[hp])

            def background(p):
                if p == 0:
                    yield from gen_qT(1)
                    yield from gen_kT(1)
                    for nt in range(NT):
                        yield from gen_v(nt)
                elif p == 1:
                    for b in (2, 3):
                        yield from gen_qT(b)
                        yield from gen_kT(b)
                elif p == 2:
                    for b in (4, 5):
                        yield from gen_qT(b)
                        yield from gen_kT(b)

            # ---- prologue + pair loop ----
            for _ in gen_qT(0):
                pass
            for _ in gen_kT(0):
                pass

            uts_by_pair = [dict() for _ in range(NPAIR)]
            av = None
            for p in range(NPAIR):
                sc = gen_scores(p, uts_by_pair[p])
                bg = background(p)
                for kt in range(NT):
                    next(sc)
                    next(sc)
                    if av is not None:
                        next(av, None)
                    next(bg, None)
                    if p == 0 and kt < 3:
                        next(bg, None)
                for _ in bg:
                    pass
                if av is not None:
                    for _ in av:
                        pass
                if p == 2:
                    s1.close()  # free x/wqkv SBUF
                av = gen_av(p, uts_by_pair[p])
            for _ in av:
                pass

            # ---- output projection ----
            for ob in range(CB):
                for qc in range(2):
                    pt = ps_mix.tile([P, 512], F32, name="pt_y", tag="mix")
                    for cb in range(CB):
                        nc.tensor.matmul(
                            pt[:],
                            wp_sb[cb][:, ob * P:(ob + 1) * P],
                            attT_sb[cb][:, qc * 512:(qc + 1) * 512],
                            start=(cb == 0), stop=(cb == CB - 1),
                        )
                    y = pool_y.tile([P, 512], F32, name="y")
                    nc.vector.tensor_scalar_add(y[:], pt[:], bias_sb[ob][:])
                    nc.sync.dma_start(
                        out_d.ap()[ob * P:(ob + 1) * P, qc * 512:(qc + 1) * 512], y[:]
                    )

    nc.compile()
    return nc


def kernel(x, w_qkv, w_proj, b_proj):
    global _cached_nc, LAST_RESULT
    if _cached_nc is None:
        _cached_nc = _build_nc()
    nc = _cached_nc

    x = np.asarray(x, dtype=np.float32)
    w_qkv = np.asarray(w_qkv, dtype=np.float32)
    w_proj = np.asarray(w_proj, dtype=np.float32)
    b_proj = np.asarray(b_proj, dtype=np.float32)

    bf = ml_dtypes.bfloat16
    wqkvT = w_qkv.T.astype(np.float32).copy()  # [C, 3C]
    wqkvT[:, :C] *= SCALE  # fold q scaling
    wqkvT = np.ascontiguousarray(wqkvT).astype(bf)
    wprojT = np.ascontiguousarray(w_proj.T).astype(bf)
    bproj_dev = np.ascontiguousarray(b_proj.astype(np.float32).reshape(CB, P, 1))

    in_maps = []
    for c in range(NCORES):
        b, half = divmod(c, 2)
        xTb = x[b].T.astype(bf)  # [C, N]
        if half:
            xTb = np.roll(xTb, -ROWS, axis=1)  # query rows -> columns 0:1024
        in_maps.append(
            {
                "xT": np.ascontiguousarray(xTb),
                "wqkvT": wqkvT,
                "wprojT": wprojT,
                "bproj": bproj_dev,
            }
        )

    res = bass_utils.run_bass_kernel_spmd(nc, in_maps, core_ids=list(range(NCORES)))
    LAST_RESULT = res

    out = np.empty((B, N, C), np.float32)
    for c in range(NCORES):
        b, half = divmod(c, 2)
        out[b, half * ROWS:(half + 1) * ROWS, :] = res.results[c]["out"].T
    return out
